# revision 17
# baseline (speedup 1.0000x reference)
"""DeltaNet forward on 8 Trainium2 NeuronCores.

Sharding: core c handles batch b = c//2 and head-pair hp = c%2 (heads
2hp, 2hp+1).  Each core computes conv + projections for its 2 heads,
runs the chunked delta-rule recurrence (chunk C=128, WY/UT transform
with a log-depth unit-triangular inverse), gated RMSNorm, and a partial
output projection out_partial = og @ Wo[head rows].  Host sums the two
partials per batch.

Chunked delta rule per (b,h), state S [DK,DV]:
    A   = Kn Kn^T; N = -beta_row o tril(A,-1)
    PT  = ((I+N)(I+N^2)...(I+N^(2^(NF-1))))^T  (= ((I-(-N))^{-1})^T, exact
          since N^(2^NF) = 0 numerically); final PT rows scaled by beta
    R   = V - Kn S0   (chunk 0: R = V)
    U   = (PT-with-beta)^T-applied: U = P diag(beta) R  via lhsT=PT
    O   = Q S0 + (triu_incl o (Kn Q^T))^T U
    S  += Kn^T U      (PSUM-resident accumulation)
"""

import numpy as np

import concourse.bass as bass
import concourse.mybir as mybir
import concourse.tile as tile
from concourse import bacc
from concourse.bass import ds
from concourse.bass_utils import run_bass_kernel_spmd

F32 = mybir.dt.float32
BF16 = mybir.dt.bfloat16
AF = mybir.ActivationFunctionType
ALU = mybir.AluOpType

B, L, D, H = 4, 2048, 1024, 4
DK, DV, KCONV, EPS = 128, 256, 4, 1e-5
C = 128          # chunk length
NKT = D // 128   # 8 contraction tiles
NF = 4           # Neumann factors: (I+N)(I+N^2)(I+N^4)(I+N^8)


def build(nc, n_chunks=L // C, nf=NF, use_silu=True):
    LL = n_chunks * C

    x = nc.dram_tensor("x", [LL, D], BF16, kind="ExternalInput")
    convd = nc.dram_tensor("convd", [128, NKT * KCONV * 128], BF16, kind="ExternalInput")
    wq = nc.dram_tensor("wq", [128, NKT * 2 * 128], BF16, kind="ExternalInput")
    wkb = nc.dram_tensor("wkb", [128, NKT * 258], BF16, kind="ExternalInput")
    wv = nc.dram_tensor("wv", [128, NKT * 512], BF16, kind="ExternalInput")
    wg = nc.dram_tensor("wg", [128, NKT * 512], BF16, kind="ExternalInput")
    wo = nc.dram_tensor("wo", [128, 4 * D], BF16, kind="ExternalInput")
    ident_d = nc.dram_tensor("ident", [128, 128], BF16, kind="ExternalInput")
    trilm_d = nc.dram_tensor("trilm", [128, 128], F32, kind="ExternalInput")
    trium_d = nc.dram_tensor("trium", [128, 128], F32, kind="ExternalInput")
    out = nc.dram_tensor("out", [LL, D], F32, kind="ExternalOutput")

    with tile.TileContext(nc) as tc:
        with (
            tc.tile_pool(name="consts", bufs=1) as consts,
            tc.tile_pool(name="hbuf", bufs=1) as hbuf,
            tc.tile_pool(name="proj", bufs=1) as proj,
            tc.tile_pool(name="small", bufs=1) as small,
            tc.tile_pool(name="scr", bufs=3) as scr,
            tc.tile_pool(name="sbf_pool", bufs=4) as sbf_pool,
            tc.tile_pool(name="ps_s", bufs=2, space="PSUM") as ps_s,
            tc.tile_pool(name="ps_proj", bufs=3, space="PSUM") as ps_proj,
            tc.tile_pool(name="ps_t", bufs=2, space="PSUM") as ps_t,
        ):
            # ---- constants / weights in SBUF ----
            cw = consts.tile([128, NKT, KCONV, 128], BF16, tag="cw")
            nc.sync.dma_start(cw, convd.rearrange("p (a b c) -> p a b c", b=KCONV, c=128))
            wq_s = consts.tile([128, NKT, 2, 128], BF16, tag="wq")
            nc.sync.dma_start(wq_s, wq.rearrange("p (a b c) -> p a b c", b=2, c=128))
            wkb_s = consts.tile([128, NKT, 258], BF16, tag="wkb")
            nc.sync.dma_start(wkb_s, wkb.rearrange("p (a b) -> p a b", b=258))
            wv_s = consts.tile([128, NKT, 512], BF16, tag="wv")
            nc.sync.dma_start(wv_s, wv.rearrange("p (a b) -> p a b", b=512))
            wg_s = consts.tile([128, NKT, 512], BF16, tag="wg")
            nc.sync.dma_start(wg_s, wg.rearrange("p (a b) -> p a b", b=512))
            wo_s = consts.tile([128, 4, D], BF16, tag="wo")
            nc.sync.dma_start(wo_s, wo.rearrange("p (a b) -> p a b", b=D))
            ident = consts.tile([128, 128], BF16, tag="ident")
            nc.sync.dma_start(ident, ident_d[:, :])
            trilm = consts.tile([128, 128], F32, tag="trilm")
            nc.sync.dma_start(trilm, trilm_d[:, :])
            trium = consts.tile([128, 128], F32, tag="trium")
            nc.sync.dma_start(trium, trium_d[:, :])

            # ---- load x transposed: xT[p, kt, l] = x[l, kt*128+p] ----
            xT = hbuf.tile([128, NKT, LL], BF16, tag="xT")
            for kt in range(NKT):
                nc.sync.dma_start_transpose(xT[:, kt, :], x[:, ds(kt * 128, 128)])

            # ---- conv (PE diagonal matmuls) + silu, written in place over xT
            # (right-to-left so the 3-col left halo is still raw x) ----
            hT = xT
            n512 = LL // 512
            for kt in range(NKT):
                for lc in range(n512 - 1, -1, -1):
                    pc = ps_t.tile([128, 512], F32, tag="t")
                    for j in range(KCONV - 1, -1, -1):
                        s = KCONV - 1 - j
                        first, last = (j == KCONV - 1), (j == 0)
                        if lc == 0 and s > 0:
                            nc.tensor.matmul(
                                pc[:, ds(s, 512 - s)], cw[:, kt, j, :],
                                xT[:, kt, ds(0, 512 - s)],
                                start=first, stop=last, skip_group_check=True)
                        else:
                            nc.tensor.matmul(
                                pc[:, :], cw[:, kt, j, :],
                                xT[:, kt, ds(lc * 512 - s, 512)],
                                start=first, stop=last, skip_group_check=True)
                    if use_silu:
                        nc.scalar.activation(hT[:, kt, ds(lc * 512, 512)],
                                             pc[:, :], AF.Silu)
                    else:
                        sgt = scr.tile([128, 512], BF16, tag="sgt")
                        nc.scalar.activation(sgt, pc[:, :], AF.Sigmoid)
                        nc.vector.tensor_mul(hT[:, kt, ds(lc * 512, 512)],
                                             pc[:, :], sgt)

            # ---- QT projections (weight-stationary): QT_all[h] [DK, LL] ----
            qt_all = proj.tile([128, 2, n_chunks, C], BF16, tag="qt")
            for h in range(2):
                for lsl in range(n512):
                    pq = ps_t.tile([128, 512], F32, tag="t")
                    for kt in range(NKT):
                        nc.tensor.matmul(pq, wq_s[:, kt, h, :],
                                         hT[:, kt, ds(lsl * 512, 512)],
                                         start=(kt == 0), stop=(kt == NKT - 1))
                    nc.scalar.activation(
                        qt_all[:, h, ds(lsl * 4, 4), :].rearrange("p a b -> p (a b)"),
                        pq, AF.Copy)

            # ---- K/beta, V, G projections (hT-stationary) ----
            k_all = proj.tile([128, n_chunks, 256], BF16, tag="ku")
            v_all = proj.tile([128, n_chunks, 512], BF16, tag="v")
            sg_all = proj.tile([128, n_chunks, 512], BF16, tag="sg")
            hb_all = small.tile([128, n_chunks, 2], F32, tag="hb")
            ssq_all = small.tile([128, n_chunks, 2], F32, tag="ssq")
            ksq_scr = scr.tile([128, 128], F32, tag="ksq")

            for c in range(n_chunks):
                pkb = ps_proj.tile([128, 258], F32, tag="prj")
                pv = ps_proj.tile([128, 512], F32, tag="prj")
                pg = ps_proj.tile([128, 512], F32, tag="prj")
                for kt in range(NKT):
                    lhs = hT[:, kt, ds(c * C, C)]
                    st, sp = (kt == 0), (kt == NKT - 1)
                    nc.tensor.matmul(pkb, lhs, wkb_s[:, kt, :], start=st, stop=sp)
                    nc.tensor.matmul(pv, lhs, wv_s[:, kt, :], start=st, stop=sp)
                    nc.tensor.matmul(pg, lhs, wg_s[:, kt, :], start=st, stop=sp)
                for h in range(2):
                    nc.scalar.activation(
                        ksq_scr, pkb[:, ds(h * 128, 128)], AF.Square,
                        accum_out=ssq_all[:, c, ds(h, 1)])
                nc.scalar.activation(k_all[:, c, :], pkb[:, 0:256], AF.Copy)
                nc.vector.tensor_copy(hb_all[:, c, :], pkb[:, 256:258])
                nc.scalar.activation(v_all[:, c, :], pv, AF.Copy)
                if use_silu:
                    nc.scalar.activation(sg_all[:, c, :], pg, AF.Silu)
                else:
                    sgt = scr.tile([128, 512], BF16, tag="sgt")
                    nc.scalar.activation(sgt, pg, AF.Sigmoid)
                    nc.vector.tensor_mul(sg_all[:, c, :], pg, sgt)

            # ---- batched norms ----
            flat = lambda t: t.rearrange("p a b -> p (a b)")
            epsk = small.tile([128, 1], F32, tag="epsk")
            nc.vector.memset(epsk, 1e-12)
            rnorm = small.tile([128, n_chunks, 2], F32, tag="rnorm")
            nc.scalar.activation(flat(rnorm), flat(ssq_all), AF.Sqrt, bias=epsk)
            nc.vector.reciprocal(flat(rnorm), flat(rnorm))
            beta_all = small.tile([128, n_chunks, 2], F32, tag="beta")
            nc.scalar.activation(flat(beta_all), flat(hb_all), AF.Sigmoid)
            negbeta = small.tile([128, n_chunks, 2], F32, tag="negbeta")
            nc.vector.tensor_scalar_mul(flat(negbeta), flat(beta_all), -1.0)

            # ---- normalize K in place, build KT ----
            kt_all = proj.tile([128, n_chunks, 2, 128], BF16, tag="kt")
            for c in range(n_chunks):
                for h in range(2):
                    kn = k_all[:, c, ds(h * 128, 128)]
                    nc.scalar.activation(kn, kn, AF.Copy,
                                         scale=rnorm[:, c, ds(h, 1)])
                    ptr = ps_t.tile([128, 128], BF16, tag="tr", bufs=1)
                    nc.tensor.transpose(ptr, kn, ident)
                    nc.vector.tensor_copy(kt_all[:, c, h, :], ptr)

            # ---- recurrence ----
            o_all = [hbuf.tile([128, n_chunks, 256], BF16, tag=f"o{h}", name=f"o{h}")
                     for h in range(2)]
            s_psum = [ps_s.tile([128, 256], F32, tag="spsum", name=f"spsum{_}")
                      for _ in range(2)]
            s_bf = [None, None]
            mo2_all = small.tile([128, n_chunks, 2], F32, tag="mo2")
            osq_scr = scr.tile([128, 256], F32, tag="osq")

            for c in range(n_chunks):
                for h in range(2):
                    ktc, qtc = kt_all[:, c, h, :], qt_all[:, h, c, :]
                    knc = k_all[:, c, ds(h * 128, 128)]
                    vc = v_all[:, c, ds(h * 256, 256)]
                    bcol = beta_all[:, c, ds(h, 1)]
                    nbcol = negbeta[:, c, ds(h, 1)]

                    # A = Kn Kn^T ; W2 = Kn Q^T
                    pA = ps_t.tile([128, 128], F32, tag="t")
                    nc.tensor.matmul(pA, ktc, ktc)
                    pW = ps_t.tile([128, 128], F32, tag="t")
                    nc.tensor.matmul(pW, ktc, qtc)
                    # N = (-beta) o tril_strict(A)
                    nmat = scr.tile([128, 128], BF16, tag="n")
                    nc.vector.scalar_tensor_tensor(
                        nmat, pA, nbcol, trilm, op0=ALU.mult, op1=ALU.mult)
                    # W2m = triu_incl o W2
                    w2m = scr.tile([128, 128], BF16, tag="w2m")
                    nc.vector.tensor_mul(w2m, pW, trium)

                    # PT build: PT0 = I + N^T
                    ptm = ps_t.tile([128, 128], BF16, tag="tr", bufs=1)
                    nc.tensor.transpose(ptm, nmat, ident)
                    mj = scr.tile([128, 128], BF16, tag="mj")
                    nc.vector.tensor_copy(mj, ptm)
                    pt_cur = scr.tile([128, 128], BF16, tag="ptc")
                    nc.vector.tensor_add(pt_cur, mj, ident)
                    tj = nmat
                    for f in range(1, nf):
                        # T_{f} = T_{f-1} @ T_{f-1}  (lhsT = M_{f-1})
                        pT = ps_t.tile([128, 128], F32, tag="t")
                        nc.tensor.matmul(pT, mj, tj)
                        tj2 = scr.tile([128, 128], BF16, tag="tj")
                        nc.scalar.activation(tj2, pT, AF.Copy)
                        # PT_f = PT_{f-1} + T_f @ PT_{f-1}
                        pP = ps_t.tile([128, 128], F32, tag="t")
                        nc.tensor.matmul(pP, tj2, pt_cur)
                        if f == nf - 1:
                            # final: PT = beta_row o (PT_prev + T @ PT_prev)
                            ptf = scr.tile([128, 128], F32, tag="ptf")
                            nc.vector.tensor_add(ptf, pP, pt_cur)
                            pt_new = scr.tile([128, 128], BF16, tag="ptc")
                            nc.scalar.activation(pt_new, ptf, AF.Copy, scale=bcol)
                        else:
                            pt_new = scr.tile([128, 128], BF16, tag="ptc")
                            nc.vector.tensor_add(pt_new, pP, pt_cur)
                        pt_cur = pt_new
                        if f < nf - 1:
                            ptm2 = ps_t.tile([128, 128], BF16, tag="tr", bufs=1)
                            nc.tensor.transpose(ptm2, tj2, ident)
                            mj = scr.tile([128, 128], BF16, tag="mj")
                            nc.vector.tensor_copy(mj, ptm2)
                            tj = tj2

                    # R = V - Kn S0 (chunk 0: R = V)
                    if c == 0:
                        rmat = vc
                    else:
                        pKS = ps_t.tile([128, 256], F32, tag="t")
                        nc.tensor.matmul(pKS, ktc, s_bf[h])
                        rmat = scr.tile([128, 256], BF16, tag="r")
                        nc.vector.scalar_tensor_tensor(
                            rmat, pKS, -1.0, vc, op0=ALU.mult, op1=ALU.add)
                    # U = P diag(beta) R
                    pU = ps_t.tile([128, 256], F32, tag="t")
                    nc.tensor.matmul(pU, pt_cur, rmat)
                    umat = scr.tile([128, 256], BF16, tag="u")
                    nc.scalar.activation(umat, pU, AF.Copy)

                    # O = Q S0 + W2m^T U
                    pO = ps_t.tile([128, 256], F32, tag="t")
                    if c == 0:
                        nc.tensor.matmul(pO, w2m, umat)
                    else:
                        nc.tensor.matmul(pO, qtc, s_bf[h], start=True, stop=False)
                        nc.tensor.matmul(pO, w2m, umat, start=False, stop=True)
                    # S += Kn^T U (PSUM resident)
                    nc.tensor.matmul(s_psum[h], knc, umat,
                                     start=(c == 0), stop=(c == n_chunks - 1),
                                     skip_group_check=True)
                    if c < n_chunks - 1:
                        sb = sbf_pool.tile([128, 256], BF16, tag=f"sbf{h}")
                        nc.scalar.activation(sb, s_psum[h], AF.Copy)
                        s_bf[h] = sb

                    # drain O + sum(o^2)
                    nc.scalar.activation(osq_scr, pO, AF.Square,
                                         accum_out=mo2_all[:, c, ds(h, 1)])
                    nc.scalar.activation(o_all[h][:, c, :], pO, AF.Copy)

            # ---- gated rmsnorm ----
            epso = small.tile([128, 1], F32, tag="epso")
            nc.vector.memset(epso, EPS)
            rms = small.tile([128, n_chunks, 2], F32, tag="rms")
            nc.scalar.activation(flat(rms), flat(mo2_all), AF.Sqrt,
                                 bias=epso, scale=1.0 / DV)
            nc.vector.reciprocal(flat(rms), flat(rms))

            ogt_all = proj.tile([128, 4, n_chunks, 128], BF16, tag="ogt")
            for c in range(n_chunks):
                for h in range(2):
                    og = scr.tile([128, 256], BF16, tag="og")
                    nc.vector.scalar_tensor_tensor(
                        og, o_all[h][:, c, :], rms[:, c, ds(h, 1)],
                        sg_all[:, c, ds(h * 256, 256)],
                        op0=ALU.mult, op1=ALU.mult)
                    for half in range(2):
                        pt2 = ps_t.tile([128, 128], BF16, tag="tr", bufs=1)
                        nc.tensor.transpose(pt2, og[:, ds(half * 128, 128)], ident)
                        nc.vector.tensor_copy(ogt_all[:, h * 2 + half, c, :], pt2)

            # ---- output projection: out[cC:(c+1)C, :] = og_c @ Wo_sel ----
            for c in range(n_chunks):
                for nh in range(2):
                    po = ps_t.tile([128, 512], F32, tag="t")
                    for kt4 in range(4):
                        nc.tensor.matmul(po, ogt_all[:, kt4, c, :],
                                         wo_s[:, kt4, ds(nh * 512, 512)],
                                         start=(kt4 == 0), stop=(kt4 == 3))
                    ost = scr.tile([128, 512], F32, tag="ost")
                    nc.scalar.activation(ost, po, AF.Copy)
                    nc.sync.dma_start(out[ds(c * C, C), ds(nh * 512, 512)], ost)
    return nc


def _prep_core_inputs(inputs, b, hp, n_chunks=L // C):
    import ml_dtypes
    bf = ml_dtypes.bfloat16
    LL = n_chunks * C
    hs = np.asarray(inputs["hidden_states"])[b, :LL]          # [LL, D]
    conv_w = np.asarray(inputs["conv_w"])                     # [D, 4]

    qcols = slice(hp * 2 * DK, hp * 2 * DK + 2 * DK)          # [256] cols of Wq/Wk
    vcols = slice(hp * 2 * DV, hp * 2 * DV + 2 * DV)          # [512] cols of Wv/Wg
    bcols = slice(hp * 2, hp * 2 + 2)

    def tile_rhs(w, ncols):
        # [D, ncols] -> [128, NKT*ncols] with w_t[p, kt, n] = w[kt*128+p, n]
        return np.ascontiguousarray(
            w.reshape(NKT, 128, ncols).transpose(1, 0, 2).reshape(128, NKT * ncols)
        ).astype(bf)

    Wq = np.asarray(inputs["Wq"])[:, qcols] * (DK ** -0.5)
    Wk = np.asarray(inputs["Wk"])[:, qcols]
    Wbeta = np.asarray(inputs["Wbeta"])[:, bcols]
    Wv = np.asarray(inputs["Wv"])[:, vcols]
    Wg = np.asarray(inputs["Wg"])[:, vcols]
    Wo = np.asarray(inputs["Wo"])[vcols, :]                   # [512, D]
    norm_w = np.asarray(inputs["norm_w"])                     # [DV]
    Wo = Wo * np.tile(norm_w, 2)[:, None]

    convd = np.zeros((128, NKT, KCONV, 128), np.float32)
    wbf = conv_w.astype(bf).astype(np.float32)
    for kt in range(NKT):
        for j in range(KCONV):
            convd[np.arange(128), kt, j, np.arange(128)] = wbf[kt * 128:(kt + 1) * 128, j]

    return {
        "x": hs.astype(bf),
        "convd": convd.reshape(128, -1).astype(bf),
        "wq": tile_rhs(Wq, 256).reshape(128, NKT, 2, 128).reshape(128, -1),
        "wkb": tile_rhs(np.concatenate([Wk, Wbeta], 1), 258),
        "wv": tile_rhs(Wv, 512),
        "wg": tile_rhs(Wg, 512),
        "wo": np.ascontiguousarray(
            Wo.reshape(4, 128, D).transpose(1, 0, 2).reshape(128, 4 * D)).astype(bf),
        "ident": np.eye(128, dtype=np.float32).astype(bf),
        "trilm": np.tril(np.ones((128, 128), np.float32), -1),
        "trium": np.triu(np.ones((128, 128), np.float32), 0),
    }


TRACE = False
LAST_EXEC_NS = None
_nc_cache = None


def kernel(**inputs):
    global _nc_cache, LAST_EXEC_NS
    if _nc_cache is None:
        nc = bacc.Bacc("TRN2", target_bir_lowering=False)
        build(nc)
        nc.compile()
        _nc_cache = nc
    in_maps = [_prep_core_inputs(inputs, c // 2, c % 2) for c in range(8)]
    r = run_bass_kernel_spmd(_nc_cache, in_maps, core_ids=list(range(8)),
                             trace=TRACE)
    LAST_EXEC_NS = r.exec_time_ns
    res = r.results
    out = np.zeros((B, L, D), np.float32)
    for b in range(B):
        out[b] = res[2 * b]["out"] + res[2 * b + 1]["out"]
    return out


if __name__ == "__main__":
    d = np.load("/root/problem/ref_data.npz")
    ins = {k: d[k] for k in ("hidden_states", "conv_w", "Wq", "Wk", "Wv",
                             "Wbeta", "Wg", "Wo", "norm_w")}
    o = kernel(**ins)
    e = d["expected"]
    print("rel err:", np.linalg.norm(o - e) / np.linalg.norm(e))
